# revision 1
# baseline (speedup 1.0000x reference)
"""Trainium2 Bass kernel for nn_AttentionTorch_77833397338547.

Computation (per batch b):
  K = keys[b,:,0,:]      [C=2048, S=1024]   (C = 16 heads x 128 head_dim)
  per head h (rows h*128:(h+1)*128 of the channel dim):
    scores[k, q] = (1/sqrt(128)) * K_h^T @ Q_h          [S, S]
    P = softmax_k(scores + mask_bias)
    hid_h[d, q]  = V_h @ P                              [128, S]
  out[o, q] = sum_c w_out[o, c] * hid[c, q]             [2048, S]

Sharding: 8 cores = (batch b in 0..3) x (query half qh in 0..1).
Each core computes the full attention + out_proj for its (b, q-slice).
No cross-core communication is needed because out_proj only mixes
channels, which stay local to a core.

Key optimizations over the f32r baseline (237.9us measured):
 1. All matmul operands stream as bf16. Measured on this hardware the PE
    runs matmuls at ~0.68 ns/moving-column (f32r: ~0.81), independent of
    weight reloads or accumulation grouping, so per-MM cost at N=512 is
    ~347ns bf16 vs ~416ns f32r. Accumulation stays fp32 in PSUM.
 2. Host-side key compaction: the attention mask keeps ~50% of the 1024
    keys (fixed-seed counts: 523/489/494/475 per batch). Valid keys are
    gathered densely on the host and padded to a multiple of 128; the
    kernel only processes ceil(max_nv/128)=5 key chunks instead of 8,
    cutting QK/AV matmuls, exp activations, and the DVE chunk-sum by
    3/8. Padded slots get bias -60 (exp -> 0) and zero K/V columns, so
    masking semantics are preserved exactly.
 3. Softmax denominator: DVE chunk-sum (kc-1 adds/head) + one
    ones-matmul per head for the cross-partition sum + broadcast.
"""

import sys

sys.path.insert(0, "/opt/trn_rl_repo")

import numpy as np

# Calibrate the Tile scheduler's cost model with rates measured on this
# hardware (bf16 matmul ~0.678 ns/moving-column; DVE ~0.775 Gelem/s/lane
# fp32-rate) so scheduling decisions reflect true engine speeds. Must run
# before the rust cost model caches the specs (first compile).
import concourse.hw_specs as _hws
import concourse.mybir as _mybir

_hws.TRN2Spec.PE_CYCLE = 0.678
_hws.TRN2Spec.CYCLE_T = dict(_hws.TRN2Spec.CYCLE_T)
_hws.TRN2Spec.CYCLE_T[_mybir.EngineType.DVE] = 1e9 / 0.775e9

B, C, S = 4, 2048, 1024
H, D = 16, 128          # heads x head_dim
QB = S // 2             # per-core query block = 512
OC = C // D             # out_proj row chunks = 16
N_CORES = 8
SCALE = 1.0 / np.sqrt(np.float32(D))
MASK_BIAS = np.float32(-60.0)
KC_FULL = S // D        # uncompacted key chunks = 8

_BUILT = {}

FAST = True   # bf16 streaming operands (validated rel l2 ~4e-3)
DVD = True    # DVE chunk-sum for the softmax denominator


def build_nc(repeat: int = 1, fast: bool = FAST, dvd: bool = DVD,
             kc: int = 5):
    """Build + compile the per-core Bass program. Cached per config.

    kc = number of 128-wide key chunks after host-side compaction.
    """
    key = (repeat, fast, dvd, kc)
    if key in _BUILT:
        return _BUILT[key]

    import concourse.bass as bass
    import concourse.mybir as mybir
    import concourse.tile as tile
    from concourse import bacc

    f32 = mybir.dt.float32
    f32r = mybir.dt.float32r
    bf16 = mybir.dt.bfloat16
    edt = bf16 if fast else f32r   # E, ones, V (attention value path)
    wdt = bf16 if fast else f32r   # w_out, hid (projection path)
    kqdt = bf16 if fast else f32r  # K, Q (score path)
    EXP = mybir.ActivationFunctionType.Exp
    SA = kc * D                    # padded compacted key count

    nc = bacc.Bacc("TRN2", target_bir_lowering=False, debug=False,
                   num_devices=N_CORES)

    k_d = nc.dram_tensor("k_in", [C, SA], kqdt, kind="ExternalInput")
    q_d = nc.dram_tensor("q_in", [C, QB], kqdt, kind="ExternalInput")
    v_d = nc.dram_tensor("v_in", [H, D, kc, D], edt, kind="ExternalInput")
    w_d = nc.dram_tensor("w_in", [OC, D, H, D], wdt, kind="ExternalInput")
    bias_d = nc.dram_tensor("bias_in", [D, kc], f32, kind="ExternalInput")
    ones_d = nc.dram_tensor("ones_in", [D, D], edt, kind="ExternalInput")
    out_d = nc.dram_tensor("out", [C, QB], f32, kind="ExternalOutput")

    def body(tc):
        with (
            tc.tile_pool(name="const", bufs=1) as const,
            tc.tile_pool(name="kvq", bufs=3) as kvq,
            tc.tile_pool(name="ep", bufs=2) as ep,
            tc.tile_pool(name="hidp", bufs=1) as hidp,
            tc.tile_pool(name="wp", bufs=3) as wp,
            tc.tile_pool(name="rcp", bufs=2) as rcp,
            tc.tile_pool(name="osb", bufs=3) as osb,
            tc.tile_pool(name="scp", bufs=3, space="PSUM") as scp,
            tc.tile_pool(name="hpp", bufs=2, space="PSUM") as hpp,
            tc.tile_pool(name="dnp", bufs=1, space="PSUM") as dnp,
            tc.tile_pool(name="opp", bufs=2, space="PSUM") as opp,
        ):
            ones_sb = const.tile([D, D], edt)
            bias_sb = const.tile([D, kc], f32)
            nc.gpsimd.dma_start(ones_sb[:], ones_d[:])
            nc.gpsimd.dma_start(bias_sb[:], bias_d[:])

            hid_all = hidp.tile([D, H, QB], wdt)
            # First NJ0 out_proj chunks get their heads-0..7 half computed
            # inside head-steps 10.. (fills PE bubbles left by cross-engine
            # pacing); ScalarE parks the halves in SBUF, the tail merges
            # them with a DVE add in place of the usual PSUM->SBUF copy.
            NJ0 = 6
            o0_list = [hidp.tile([D, QB], f32, name=f"o0_{j}")
                       for j in range(NJ0)]

            def emit_op_half0(j):
                w_sb = wp.tile([D, 8, D], wdt, tag="wh0")
                nc.gpsimd.dma_start(w_sb[:], w_d[j, :, 0:8, :])
                op = opp.tile([D, QB], f32)
                for cc in range(8):
                    nc.tensor.matmul(op[:], w_sb[:, cc, :], hid_all[:, cc, :],
                                     start=(cc == 0), stop=(cc == 7))
                nc.scalar.copy(o0_list[j][:], op[:])

            # Software-pipelined head loop (depth 2). Per step h the PE
            # emission order is [QK_h x kc, dn_{h-2}, AV_{h-1} x kc], so
            # every PE instruction has ~a full head of slack w.r.t. the
            # cross-engine producers it waits on (exp on ACT, chunk-sum on
            # DVE) and PE never stalls on handoff latency.
            def emit_av_chain(prev):
                v_sb, e_sb, h = prev
                hp = hpp.tile([D, QB], f32)
                for c in range(kc):
                    nc.tensor.matmul(hp[:], v_sb[:, c, :], e_sb[:, c, :],
                                     start=(c == 0), stop=(c == kc - 1))
                acc = ep.tile([D, QB], edt, tag="dv")
                nc.vector.tensor_add(acc[:], e_sb[:, 0, :], e_sb[:, 1, :])
                for c in range(2, kc):
                    nxt = ep.tile([D, QB], edt, tag="dv")
                    nc.vector.tensor_add(nxt[:], acc[:], e_sb[:, c, :])
                    acc = nxt
                return (acc, hp, h)

            def flush_dn(pend):
                acc, hp, h = pend
                dn = dnp.tile([D, QB], f32)
                nc.tensor.matmul(dn[:], ones_sb[:], acc[:],
                                 start=True, stop=True)
                rc = rcp.tile([D, QB], f32)
                nc.vector.reciprocal(rc[:], dn[:])
                nc.vector.tensor_mul(hid_all[:, h, :], hp[:], rc[:])

            prev = None   # (v_sb, e_sb, h) awaiting AV + chunk-sum
            pend = None   # (acc, hp, h) awaiting dn matmul + normalize
            for h in range(H):
                k_sb = kvq.tile([D, SA], kqdt)
                q_sb = kvq.tile([D, QB], kqdt)
                v_sb = kvq.tile([D, kc, D], edt)
                nc.sync.dma_start(k_sb[:], k_d[h * D:(h + 1) * D, :])
                nc.sync.dma_start(q_sb[:], q_d[h * D:(h + 1) * D, :])
                nc.gpsimd.dma_start(v_sb[:], v_d[h])

                e_sb = ep.tile([D, kc, QB], edt)
                for c in range(kc):
                    sc = scp.tile([D, QB], f32)
                    nc.tensor.matmul(sc[:], k_sb[:, c * D:(c + 1) * D], q_sb[:],
                                     start=True, stop=True)
                    nc.scalar.activation(e_sb[:, c, :], sc[:], EXP,
                                         bias=bias_sb[:, c:c + 1], scale=1.0)

                if pend is not None:
                    flush_dn(pend)
                    pend = None
                if prev is not None:
                    pend = emit_av_chain(prev)
                prev = (v_sb, e_sb, h)
                if 10 <= h < 10 + NJ0:
                    emit_op_half0(h - 10)

            if pend is not None:
                flush_dn(pend)
            flush_dn(emit_av_chain(prev))

            for j in range(OC):
                if j < NJ0:
                    w_sb = wp.tile([D, 8, D], wdt, tag="wh1")
                    nc.gpsimd.dma_start(w_sb[:], w_d[j, :, 8:16, :])
                    op = opp.tile([D, QB], f32)
                    for cc in range(8):
                        nc.tensor.matmul(op[:], w_sb[:, cc, :],
                                         hid_all[:, 8 + cc, :],
                                         start=(cc == 0), stop=(cc == 7))
                    o_sb = osb.tile([D, QB], f32)
                    nc.vector.tensor_add(o_sb[:], op[:], o0_list[j][:])
                else:
                    w_sb = wp.tile([D, H, D], wdt)
                    nc.gpsimd.dma_start(w_sb[:], w_d[j])
                    op = opp.tile([D, QB], f32)
                    for cc in range(H):
                        nc.tensor.matmul(op[:], w_sb[:, cc, :],
                                         hid_all[:, cc, :],
                                         start=(cc == 0), stop=(cc == H - 1))
                    o_sb = osb.tile([D, QB], f32)
                    nc.vector.tensor_copy(o_sb[:], op[:])
                nc.sync.dma_start(out_d[j * D:(j + 1) * D, :], o_sb[:])

    with tile.TileContext(nc) as tc:
        if repeat == 1:
            body(tc)
        else:
            PE = mybir.EngineType.PE
            ACT = mybir.EngineType.Activation
            DVE = mybir.EngineType.DVE
            SP = mybir.EngineType.SP
            with tc.For_i(0, repeat, 1, hint_engines=(PE, ACT, DVE, SP)):
                body(tc)

    nc.compile()
    _BUILT[key] = nc
    return nc


def compute_kc(attention_mask) -> int:
    """Key chunks needed after compaction: ceil(max_valid/128), >=1."""
    mask = np.asarray(attention_mask)
    max_nv = int(mask.reshape(B, S).sum(axis=1).max())
    return max(1, (max_nv + D - 1) // D)


def shard_inputs(keys, values, queries, attention_mask, w_out, fast=None,
                 kc=None):
    """Host-side prep: compact keys by mask, slice per core, pre-layout."""
    if fast is None:
        fast = FAST
    if fast:
        import ml_dtypes
        vdt = wdt = kqdt = ml_dtypes.bfloat16
    else:
        vdt = wdt = kqdt = np.float32
    keys = np.ascontiguousarray(np.asarray(keys, dtype=np.float32))
    values = np.ascontiguousarray(np.asarray(values, dtype=np.float32))
    queries = np.asarray(queries, dtype=np.float32)
    mask = np.asarray(attention_mask)
    w_out = np.asarray(w_out, dtype=np.float32)
    if kc is None:
        kc = compute_kc(mask)
    SA = kc * D

    # w_host[j, p, cc, o] = w_out[j*128+o, cc*128+p]; shared by all cores
    w_host = np.ascontiguousarray(
        w_out.reshape(OC, D, H, D).transpose(0, 3, 2, 1)).astype(wdt)
    ones = np.ones((D, D), dtype=vdt)

    in_maps = []
    comp = {}
    for b in range(B):
        idx = np.flatnonzero(mask[b])
        nv = len(idx)
        assert 0 < nv <= SA, (nv, SA)
        kb = np.zeros((C, SA), dtype=np.float32)
        kb[:, :nv] = keys[b, :, 0, idx].T      # fancy-index gathers -> [nv, C]
        vb_f = np.zeros((C, SA), dtype=np.float32)
        vb_f[:, :nv] = values[b, :, 0, idx].T
        # v_host[h, p, c, d] = vb_f[h*128+d, c*128+p]
        vb = np.ascontiguousarray(
            vb_f.reshape(H, D, kc, D).transpose(0, 3, 2, 1)).astype(vdt)
        bias = np.full(SA, MASK_BIAS, dtype=np.float32)
        bias[:nv] = 0.0
        bias = np.ascontiguousarray(bias.reshape(kc, D).T)  # [D, kc]
        comp[b] = (kb.astype(kqdt), vb, bias)

    for core in range(N_CORES):
        b, qh = core // 2, core % 2
        kb, vb, bias = comp[b]
        qb = (np.ascontiguousarray(
            queries[b, :, 0, qh * QB:(qh + 1) * QB]) * SCALE).astype(kqdt)
        in_maps.append({
            "k_in": kb, "q_in": qb, "v_in": vb,
            "w_in": w_host, "bias_in": bias, "ones_in": ones,
        })
    return in_maps


def kernel(keys, values, queries, attention_mask, w_out):
    from concourse.bass_utils import run_bass_kernel_spmd

    kc = compute_kc(attention_mask)
    nc = build_nc(repeat=1, fast=FAST, dvd=DVD, kc=kc)
    in_maps = shard_inputs(keys, values, queries, attention_mask, w_out,
                           fast=FAST, kc=kc)
    res = run_bass_kernel_spmd(nc, in_maps, list(range(N_CORES)))

    out = np.empty((B, C, 1, S), dtype=np.float32)
    for core in range(N_CORES):
        b, qh = core // 2, core % 2
        out[b, :, 0, qh * QB:(qh + 1) * QB] = res.results[core]["out"]
    return out

